# revision 31
# baseline (speedup 1.0000x reference)
"""Causal self-attention (B=2, T=4096, C=768, H=12, Dh=64) on 8 TRN2 NeuronCores.

Sharding: batch x head-groups. Core c handles batch b = c//4 and the 3 heads
hh = 3*(c%4) .. hh+2 of that batch (data parallel on B, tensor parallel on
heads for the qkv / out projections). Each core computes a partial output
y_c = attn_out(heads) @ W_out[head rows]; the host sums the 4 partials per
batch and adds b_out.

All matmul operands are bf16 (host pre-converts x / W_qkv / W_out); PSUM
accumulation stays fp32, y is written bf16 and upcast on the host. bf16
halves the DMA traffic and runs matmuls at 1 cycle/row.

Device-side layout (per core, identical SPMD program):
  xt    [768, 4096]  x[b].T bf16 (C on partitions)
  wqkv  [768, 576]   bf16, columns permuted to [q0 q1 | k0 k1 | q2 k2 | v0 v1 v2]
  bqkv  [576]        f32, same permutation
  wout  [192, 768]   bf16, rows for this core's heads
  y     [4096, 768]  bf16 partial output (no b_out)

Projection phase produces bf16:
  A  = [q0|q1]^T  [128, T]   (head0 on partitions 0-63, head1 on 64-127)
  B0 = [k0|0], B1 = [0|k1], D = [k2|0]   zero-padded k tiles [128, T]
  Cc = [q2|k2]^T  [128, T]
  v_st [128, 32, 3, 65]      v in [token, d] layout per 128-token block,
                             col 64 = 1.0 (softmax row-sums for free)

K=64 matmuls run at HALF rate on real TRN2 (measured via probes.py; the
CoreSim cost model misses this), so every s^T contraction is zero-padded to
128 partitions: lhsT = padded k tile (the zero half kills the other head's
rows in the packed q rhs), rhs = A or Cc full-height. Same trick pads the
out-projection's at2/wo2 pair.

Attention per (q-super of 1024, head): s^T tiles [128 k, 512 q] via
matmul(lhsT=k_pad, rhs=q_packed) into PSUM, exp'd into bf16 pt tiles either
on ACT (exact exp, scale=1/8, no max subtraction: logits ~N(0,1)) or on DVE
via int16 Schraudolph (bitcast bf16, ~3% err; never on qs=0 where diagonal
mass dominates). A static greedy balancer with hardware-calibrated per-op
costs splits the exp chunks + evacuations between ACT and DVE. The PE
stream is software-pipelined: av matmuls for round kb are emitted after the
s matmuls of round kb+lag so the in-order PE never stalls on an exp, and
the pipeline-drain bubbles at head boundaries are filled with the previous
super's out-projection tiles. Supers run in order 1,2,3,0 so the ACT-bound
qs=0 overlaps super 3's out-projection.

av^T accumulates matmul(lhsT=v_aug, rhs=p^T) over k-blocks into two
[65, 512] PSUM halves; the low half stops accumulating early (last_r0) and
is evacuated while the high half still runs, hiding the PSUM WAR on the
next head. Row 64 is the softmax denominator; normalize per 512-col half
with reciprocal_approx_fast + gpsimd partition-broadcast + DVE multiply
into bf16 attnT. Out-projection contracts attnT (2x K=128 matmuls) with
wout, evacuating per 512-col half straight to a bf16 y tile.
"""

import math

import numpy as np

import concourse.bass as bass
import concourse.tile as tile
from concourse import bacc, mybir
from concourse.bass_utils import run_bass_kernel_spmd

F32 = mybir.dt.float32
F32R = mybir.dt.float32r
BF16 = mybir.dt.bfloat16
I16 = mybir.dt.int16
I32 = mybir.dt.int32

# Schraudolph exp constants, int16 domain (bf16 = top 16 bits of f32):
# exp(s*SCALE) ~= bitcast_bf16(int16(s*A16 + B16))
LOG2E = 1.4426950408889634
SCH_A16 = 128.0 * LOG2E  # * SCALE applied at use site
SCH_B16 = 128.0 * (127.0 - 0.04367744890362246)

T = 4096
C = 768
H = 12
DH = 64
HPC = 3  # heads per core
NCORES = 8
SUP = 1024  # q-super width
NSUP = T // SUP
KB = 128  # k-block
NKB = T // KB
CH = 512  # st psum chunk / exp granularity
SCALE = 1.0 / math.sqrt(DH)

TRACE = False
LAST_RESULT = None
_PROG = None


DEFAULT_CFG = {
    "lag": 2,  # av pipeline depth in rounds behind s
    "st_bufs": 6,
    "av_bufs": 2,
    "pt_bufs": 4,
    "nrm_bufs": 2,
    "dve_exp": 1,  # allow DVE int16-Schraudolph for non-diagonal chunks
    "pool_tt": 0,  # run SBUF-only mask/norm multiplies on gpsimd (Pool)
    "repeat": 1,  # hardware For_i repetitions of the body (benchmarking)
}

# estimated per-op engine costs (ns) for the static ACT/DVE balancer,
# calibrated against hardware microbenchmarks (probes.py): 512-col exp on
# ACT = 638ns, int16 Schraudolph on DVE = 637ns, bf16 tt on DVE = 346ns
_ACT_COL = 0.833
_DVE_COL = 1.0417
_ACT_PSUM = 212.0
_DVE_PSUM = 104.0
_DVE_SBUF = 80.0


class _Balancer:
    def __init__(self):
        self.act = 0.0
        self.dve = 0.0

    def pick(self, act_cost, dve_cost, act_dve_extra=0.0):
        """Pick the engine that finishes this op sooner; update its load.
        act_dve_extra: DVE-side cost incurred when the ACT variant is chosen
        (e.g. the maskadd that precedes an ACT diagonal exp)."""
        if self.act + act_cost <= self.dve + dve_cost:
            self.act += act_cost
            self.dve += act_dve_extra
            return "act"
        self.dve += dve_cost
        return "dve"


def build_program(debug=False, cfg=None):
    cfg = {**DEFAULT_CFG, **(cfg or {})}
    nc = bacc.Bacc("TRN2", target_bir_lowering=False, debug=False)
    xt_d = nc.dram_tensor("xt", [C, T], BF16, kind="ExternalInput").ap()
    wqkv_d = nc.dram_tensor("wqkv", [C, 576], BF16, kind="ExternalInput").ap()
    bqkv_d = nc.dram_tensor("bqkv", [576], F32, kind="ExternalInput").ap()
    wout_d = nc.dram_tensor("wout", [192, C], BF16, kind="ExternalInput").ap()
    y_d = nc.dram_tensor("y", [T, C], BF16, kind="ExternalOutput").ap()

    with tile.TileContext(nc) as tc:
        with tc.tile_pool(name="res", bufs=1) as res:
            # K=64 matmuls run at HALF rate on real TRN2 hardware, so every
            # s^T lhsT is zero-padded to a full 128-partition contraction:
            # the zero half kills the other head's rows in the packed rhs.
            A = res.tile([128, T], BF16, tag="A")  # [q0 | q1] packed
            B0 = res.tile([128, T], BF16, tag="B0")  # [k0 | 0]
            B1 = res.tile([128, T], BF16, tag="B1")  # [0 | k1]
            Cc = res.tile([128, T], BF16, tag="Cc")  # [q2 | k2] packed
            D = res.tile([128, T], BF16, tag="D")  # [k2 | 0]
            v_st = res.tile([128, NKB, HPC, DH + 1], BF16, tag="v_st")
            wo01 = res.tile([128, C], BF16, tag="wo01")
            wo2 = res.tile([128, C], BF16, tag="wo2")  # rows 64-127 zero
            at01 = res.tile([128, T], BF16, tag="at01")  # [h0 d | h1 d] x q
            at2 = res.tile([128, T], BF16, tag="at2")  # rows 64-127 zero
            nc.vector.memset(B0[64:128, :], 0.0)
            nc.vector.memset(B1[0:64, :], 0.0)
            nc.gpsimd.memset(D[64:128, :], 0.0)
            nc.gpsimd.memset(at2[64:128, :], 0.0)
            nc.vector.memset(wo2[64:128, :], 0.0)

            # multiplicative causal mask for the diagonal 128-block
            # (1.0 where q >= k else 0.0), bf16 for 2x DVE
            mask01 = res.tile([128, KB], BF16, tag="mask")
            nc.gpsimd.memset(mask01[:], 1.0)
            nc.gpsimd.affine_select(
                out=mask01[:],
                in_=mask01[:],
                compare_op=mybir.AluOpType.is_ge,
                fill=0.0,
                base=0,
                pattern=[[1, KB]],
                channel_multiplier=-1,
            )
            # additive variant (0 / -1e5) for diagonal tiles kept on ACT
            maskadd = res.tile([128, KB], F32, tag="maskadd")
            nc.gpsimd.memset(maskadd[:], 0.0)
            nc.gpsimd.affine_select(
                out=maskadd[:],
                in_=maskadd[:],
                compare_op=mybir.AluOpType.is_ge,
                fill=-1e5,
                base=0,
                pattern=[[1, KB]],
                channel_multiplier=-1,
            )
            nc.vector.memset(v_st[:, :, :, DH : DH + 1], 1.0)

            import contextlib

            rep_ctx = (
                tc.For_i(0, cfg["repeat"], 1)
                if cfg.get("repeat", 1) > 1
                else contextlib.nullcontext()
            )
            rep_ctx.__enter__()

            # ---------------- Phase 1: projections ----------------
            with (
                tc.tile_pool(name="p1", bufs=1) as p1,
                tc.tile_pool(name="xts", bufs=3) as xpool,
                tc.tile_pool(name="pps", bufs=2, space="PSUM") as pps,
                tc.tile_pool(name="vps", bufs=2, space="PSUM") as vps,
            ):
                wq_sb = p1.tile([128, 6, 576], BF16, tag="wq")
                bias_qk = p1.tile([128, 3], F32, tag="bqk")
                bias_v = p1.tile([128, 192], F32, tag="bv")
                bias_v_row = p1.tile([1, 192], F32, tag="bvr")

                def load_xts(ts):
                    xts = xpool.tile([128, 6, 512], BF16, tag="xts")
                    # two half-chunks: the ci=0..2 matmuls start after the
                    # first half lands
                    for half in range(2):
                        nc.sync.dma_start(
                            xts[:, half * 3 : (half + 1) * 3, :],
                            xt_d[
                                half * 384 : (half + 1) * 384,
                                ts * 512 : (ts + 1) * 512,
                            ].rearrange("(ci p) n -> p ci n", p=128),
                        )
                    return xts

                # startup order: weight chunk ci=0, then the first x chunk,
                # then the rest — the first matmul needs only ci=0 + x half 0
                nc.sync.dma_start(wq_sb[:, 0, :], wqkv_d[0:128, :])
                xts0 = load_xts(0)
                for ci in range(1, 6):
                    nc.sync.dma_start(
                        wq_sb[:, ci, :],
                        wqkv_d[ci * 128 : (ci + 1) * 128, :],
                    )
                nc.sync.dma_start(
                    bias_qk[:],
                    bqkv_d[0:384].rearrange("(m p) -> p m", p=128),
                )
                nc.sync.dma_start(
                    bias_v_row[0:1, :],
                    bqkv_d[384:576].rearrange("(b f) -> b f", b=1),
                )
                # wout is needed only in phase 2 — load it behind everything
                nc.sync.dma_start(wo01[:], wout_d[0:128, :])
                nc.sync.dma_start(wo2[0:64, :], wout_d[128:192, :])
                nc.gpsimd.partition_broadcast(bias_v[:], bias_v_row[0:1, :])

                for ts in range(T // 512):
                    xts = xts0 if ts == 0 else load_xts(ts)
                    col0 = ts * 512
                    # q/k rows (transposed layout): psum [qkv-rows, tokens]
                    for m in range(3):
                        psq = pps.tile([128, 512], F32, tag="psq")
                        for ci in range(6):
                            nc.tensor.matmul(
                                psq[:],
                                wq_sb[:, ci, m * 128 : (m + 1) * 128],
                                xts[:, ci, :],
                                start=(ci == 0),
                                stop=(ci == 5),
                            )
                        if m == 1:
                            # k0/k1 land in separate zero-padded tiles
                            nc.vector.tensor_scalar_add(
                                out=B0[0:64, col0 : col0 + 512],
                                in0=psq[0:64, :],
                                scalar1=bias_qk[0:64, m : m + 1],
                            )
                            nc.vector.tensor_scalar_add(
                                out=B1[64:128, col0 : col0 + 512],
                                in0=psq[64:128, :],
                                scalar1=bias_qk[64:128, m : m + 1],
                            )
                        else:
                            dest = A if m == 0 else Cc
                            nc.vector.tensor_scalar_add(
                                out=dest[:, col0 : col0 + 512],
                                in0=psq[:],
                                scalar1=bias_qk[:, m : m + 1],
                            )
                    # v in [token, d] layout: psum [tokens, 3*64]
                    for tb in range(4):
                        psv = vps.tile([128, 192], F32, tag="psv")
                        for ci in range(6):
                            nc.tensor.matmul(
                                psv[:],
                                xts[:, ci, tb * 128 : (tb + 1) * 128],
                                wq_sb[:, ci, 384:576],
                                start=(ci == 0),
                                stop=(ci == 5),
                            )
                        kb = ts * 4 + tb
                        nc.vector.tensor_tensor(
                            out=v_st[:, kb, :, 0:DH],
                            in0=psv[:].rearrange("p (h d) -> p h d", h=HPC),
                            in1=bias_v[:].rearrange("p (h d) -> p h d", h=HPC),
                            op=mybir.AluOpType.add,
                        )

            # k2 lives at partitions 64-127 of Cc; shift it down to D's rows
            # 0-63 (rows 64-127 of D stay zero) so h2's s matmul contracts a
            # full 128 partitions against the packed Cc rhs.
            nc.sync.dma_start(D[0:64, :], Cc[64:128, :])

            # ---------------- Phase 2: attention + out-projection ----------------
            bal = _Balancer()
            with (
                tc.tile_pool(name="stps", bufs=cfg["st_bufs"], space="PSUM") as stps,
                tc.tile_pool(name="avps", bufs=cfg["av_bufs"], space="PSUM") as avps,
                tc.tile_pool(name="ptp", bufs=cfg["pt_bufs"]) as ptp,
                tc.tile_pool(name="nrm", bufs=cfg["nrm_bufs"]) as nrm,
                tc.tile_pool(name="ysb", bufs=4) as ypool,
            ):

                def q_ap(h, kb):
                    # packed rhs: the other head's rows are killed by the
                    # zero half of the padded k lhsT
                    return Cc[:, :] if h == 2 else A[:, :]

                def k_ap(h, kb):
                    return (B0, B1, D)[h][:, :]

                def s_exp_round(qs, h, kb):
                    """s^T matmuls + exp into a bf16 pt tile for one
                    (q-super, head, k-block) round. Returns (pt, ext0)."""
                    q0 = qs * SUP
                    t = kb - qs * (SUP // KB)  # >= 0 on the diagonal
                    ext0 = max(t, 0) * KB
                    pt = ptp.tile([128, SUP], BF16, tag="pt")
                    for ch0 in range(0, SUP, CH):
                        ch1 = ch0 + CH
                        lo = max(ch0, ext0)
                        if lo >= ch1:
                            continue
                        n = ch1 - lo
                        st = stps.tile([128, CH], F32, tag="st")
                        nc.tensor.matmul(
                            st[:, lo - ch0 : CH],
                            k_ap(h, kb)[:, kb * KB : (kb + 1) * KB],
                            q_ap(h, kb)[:, q0 + lo : q0 + ch1],
                            start=True,
                            stop=True,
                        )
                        has_diag = t >= 0 and ch0 <= ext0 < ch1
                        if t >= 0 and qs == 0:
                            # qs=0: exact exp on ACT only (softmax there is
                            # mostly diagonal mass; Schraudolph's ~3% error
                            # would survive). Additive mask before exp.
                            eng = "act"
                        elif not cfg["dve_exp"]:
                            eng = "act"
                        else:
                            act_cost = n * _ACT_COL + _ACT_PSUM
                            dve_cost = n * _DVE_COL + _DVE_PSUM
                            mask_cost = 0.0
                            if has_diag:
                                # ACT variant needs a DVE maskadd; DVE variant
                                # a bf16 mask multiply
                                mask_cost = KB * _DVE_COL + _DVE_PSUM
                                dve_cost += KB * _DVE_COL * 0.5 + _DVE_SBUF
                            eng = bal.pick(act_cost, dve_cost, mask_cost)
                        if eng == "act":
                            if has_diag:
                                nc.vector.tensor_tensor(
                                    out=st[:, ext0 - ch0 : ext0 - ch0 + KB],
                                    in0=st[:, ext0 - ch0 : ext0 - ch0 + KB],
                                    in1=maskadd[:],
                                    op=mybir.AluOpType.add,
                                )
                            if t >= 0 and qs == 0:
                                bal.act += n * _ACT_COL + _ACT_PSUM
                                if has_diag:
                                    bal.dve += KB * _DVE_COL + _DVE_PSUM
                            nc.scalar.activation(
                                out=pt[:, lo:ch1],
                                in_=st[:, lo - ch0 : CH],
                                func=mybir.ActivationFunctionType.Exp,
                                bias=0.0,
                                scale=SCALE,
                            )
                        else:
                            nc.vector.tensor_scalar(
                                out=pt[:, lo:ch1].bitcast(I16),
                                in0=st[:, lo - ch0 : CH],
                                scalar1=float(SCH_A16 * SCALE),
                                scalar2=float(SCH_B16),
                                op0=mybir.AluOpType.mult,
                                op1=mybir.AluOpType.add,
                            )
                            if has_diag:
                                # SBUF-only multiply — gpsimd (Pool) keeps it
                                # off the exp engines when pool_tt is set
                                eng2 = nc.gpsimd if cfg["pool_tt"] else nc.vector
                                if not cfg["pool_tt"]:
                                    bal.dve += KB * _DVE_COL * 0.5 + _DVE_SBUF
                                eng2.tensor_tensor(
                                    out=pt[:, ext0 : ext0 + KB],
                                    in0=pt[:, ext0 : ext0 + KB],
                                    in1=mask01[:],
                                    op=mybir.AluOpType.mult,
                                )
                    return pt, ext0

                def av_round(qs, h, kb, pt, ext0, avA, avB, stg):
                    """Accumulate av^T += v_aug^T @ p^T for one round. av is
                    split into two [65,512] PSUM tiles; each half is
                    evacuated to stg as soon as its accumulation stops (the
                    low half stops early), hiding the WAR on the next head's
                    av tiles."""
                    nkb = (qs + 1) * (SUP // KB)
                    last_r0 = qs * 8 + 512 // KB - 1  # last kb touching [0,512)
                    c = ext0
                    while c < SUP:
                        ce = min((c // 512 + 1) * 512, SUP)
                        half = c // 512
                        av = avA if half == 0 else avB
                        stop_kb = last_r0 if ce <= 512 else nkb - 1
                        nc.tensor.matmul(
                            av[:, c - half * 512 : ce - half * 512],
                            v_st[:, kb, h, :],
                            pt[:, c:ce],
                            start=(kb == 0),
                            stop=(kb == stop_kb),
                        )
                        if kb == stop_kb:
                            n = 512
                            eng = bal.pick(
                                n * _ACT_COL + _ACT_PSUM, n * _DVE_COL + _DVE_PSUM
                            )
                            dst = stg[:, half * 512 : (half + 1) * 512]
                            if eng == "act":
                                nc.scalar.copy(dst, av[:])
                            else:
                                nc.vector.tensor_copy(dst, av[:])
                        c = ce

                def norm_head(qs, h, stg):
                    """rows 0-63 of stg divided by row 64, into attnT storage.
                    reciprocal_approx_fast misreads PSUM at partition offset
                    64, so this works on the SBUF stage; the denominator row
                    is DMA-shifted to partition 0 first."""
                    q0 = qs * SUP
                    l0 = nrm.tile([1, SUP], F32, tag="l0")
                    rec = nrm.tile([1, SUP], F32, tag="rec")
                    recb = nrm.tile([64, SUP], F32, tag="recb")
                    if h == 1:
                        # h1 rows belong at partitions 64-127 of at01; DVE
                        # can't shift partitions, so stage + DMA.
                        h1s = nrm.tile([64, SUP], BF16, tag="h1stage")
                    # per-512-half chain so the first at01 half (and with it
                    # the first out-projection tiles) unblocks early
                    for c0 in (0, SUP // 2):
                        c1 = c0 + SUP // 2
                        nc.sync.dma_start(l0[0:1, c0:c1], stg[64:65, c0:c1])
                        nc.vector.reciprocal_approx_fast(
                            out=rec[0:1, c0:c1], in_=l0[0:1, c0:c1]
                        )
                        nc.gpsimd.partition_broadcast(
                            recb[:, c0:c1], rec[0:1, c0:c1]
                        )
                        if h == 0:
                            dest = at01[0:64, q0 + c0 : q0 + c1]
                        elif h == 2:
                            dest = at2[0:64, q0 + c0 : q0 + c1]
                        else:
                            dest = h1s[:, c0:c1]
                        # SBUF-only multiply — gpsimd, keeping DVE free for exps
                        eng2 = nc.gpsimd if cfg["pool_tt"] else nc.vector
                        eng2.tensor_tensor(
                            out=dest,
                            in0=stg[0:64, c0:c1],
                            in1=recb[:, c0:c1],
                            op=mybir.AluOpType.mult,
                        )
                        if h == 1:
                            nc.sync.dma_start(
                                at01[64:128, q0 + c0 : q0 + c1], h1s[:, c0:c1]
                            )
                    bal.dve += (1.0 if cfg["pool_tt"] else 2.0) * (
                        SUP * _DVE_COL
                    ) + 2 * _DVE_PSUM + 4 * _DVE_SBUF

                def oproj_tb(qs, tb):
                    """One 128-token tile of the out-projection for super qs.
                    Each 512-col half is evacuated and DMA'd independently so
                    the PSUM tile and y_sb recycle fast."""
                    tcol = qs * SUP + tb * 128
                    y_sb = ypool.tile([128, C], BF16, tag="ysb")
                    for rs, re in ((0, 512), (512, C)):
                        yps = stps.tile([128, CH], F32, tag="st")
                        nc.tensor.matmul(
                            yps[:, 0 : re - rs],
                            at01[:, tcol : tcol + 128],
                            wo01[:, rs:re],
                            start=True,
                            stop=False,
                        )
                        nc.tensor.matmul(
                            yps[:, 0 : re - rs],
                            at2[:, tcol : tcol + 128],
                            wo2[:, rs:re],
                            start=False,
                            stop=True,
                        )  # at2 rows 64-127 are zero: full-K at 1c/row
                        n = re - rs
                        eng = bal.pick(
                            n * _ACT_COL + _ACT_PSUM, n * _DVE_COL + _DVE_PSUM
                        )
                        if eng == "act":
                            nc.scalar.copy(y_sb[:, rs:re], yps[:, 0 : re - rs])
                        else:
                            nc.vector.tensor_copy(y_sb[:, rs:re], yps[:, 0 : re - rs])
                        nc.sync.dma_start(
                            y_d[tcol : tcol + 128, rs:re], y_sb[:, rs:re]
                        )

                # Supers processed in order 1,2,3,0: qs=0 is PE-light but
                # ACT-bound (exact-exp only), so overlapping it with super
                # 3's out-projection keeps the PE fed and shrinks the tail.
                # Out-projection tiles of the previously processed super are
                # slotted after each head (fills the PE bubble while the norm
                # chain and the last exps drain).
                SUPER_ORDER = (1, 2, 3, 0)
                OPROJ_SLOTS = {0: (0, 1, 2), 1: (3, 4, 5), 2: (6, 7)}
                LAG = cfg["lag"]

                for i, qs in enumerate(SUPER_ORDER):
                    prev = SUPER_ORDER[i - 1] if i >= 1 else None
                    nkb = (qs + 1) * (SUP // KB)
                    for h in range(HPC):
                        avA = avps.tile([65, SUP // 2], F32, tag="av")
                        avB = avps.tile([65, SUP // 2], F32, tag="av")
                        stg = nrm.tile([65, SUP], F32, tag="avstage")
                        slots = list(OPROJ_SLOTS[h]) if prev is not None else []
                        pend = []
                        for kb in range(nkb):
                            pend.append((kb, *s_exp_round(qs, h, kb)))
                            if len(pend) > LAG:
                                kb0, pt0, e0 = pend.pop(0)
                                av_round(qs, h, kb0, pt0, e0, avA, avB, stg)
                        while pend:
                            kb0, pt0, e0 = pend.pop(0)
                            av_round(qs, h, kb0, pt0, e0, avA, avB, stg)
                            # fill the pipeline-drain bubble (the last avs
                            # wait on exps) with out-projection PE work
                            if slots:
                                oproj_tb(prev, slots.pop(0))
                        norm_head(qs, h, stg)
                        for tb in slots:
                            oproj_tb(prev, tb)
                for tb in range(SUP // 128):
                    oproj_tb(SUPER_ORDER[-1], tb)
            rep_ctx.__exit__(None, None, None)

    nc.compile()
    return nc


def shard_inputs(x, W_qkv, b_qkv, W_out, b_out):
    """Build the per-core input maps (host-side sharding, bf16 weights/x)."""
    import ml_dtypes

    bf16 = ml_dtypes.bfloat16
    x = np.asarray(x, dtype=np.float32)
    W_qkv = np.asarray(W_qkv, dtype=np.float32)
    b_qkv = np.asarray(b_qkv, dtype=np.float32)
    W_out = np.asarray(W_out, dtype=np.float32)
    in_maps = []
    for c in range(NCORES):
        b = c // 4
        hh = (c % 4) * HPC
        h0, h1, h2 = hh, hh + 1, hh + 2

        def qcols(h):
            return list(range(h * DH, (h + 1) * DH))

        def kcols(h):
            return list(range(C + h * DH, C + (h + 1) * DH))

        def vcols(h):
            return list(range(2 * C + h * DH, 2 * C + (h + 1) * DH))

        perm = (
            qcols(h0) + qcols(h1) + kcols(h0) + kcols(h1) + qcols(h2) + kcols(h2)
            + vcols(h0) + vcols(h1) + vcols(h2)
        )
        in_maps.append(
            {
                "xt": np.ascontiguousarray(x[b].T).astype(bf16),
                "wqkv": np.ascontiguousarray(W_qkv[:, perm]).astype(bf16),
                "bqkv": np.ascontiguousarray(b_qkv[perm]),
                "wout": np.ascontiguousarray(
                    W_out[hh * DH : (hh + HPC) * DH, :]
                ).astype(bf16),
            }
        )
    return in_maps


def kernel(x, W_qkv, b_qkv, W_out, b_out):
    global _PROG, LAST_RESULT
    if _PROG is None:
        _PROG = build_program()
    nc = _PROG
    in_maps = shard_inputs(x, W_qkv, b_qkv, W_out, b_out)
    res = run_bass_kernel_spmd(nc, in_maps, list(range(NCORES)), trace=TRACE)
    LAST_RESULT = res
    b_out = np.asarray(b_out, dtype=np.float32)
    y = np.zeros((2, T, C), dtype=np.float32)
    for c in range(NCORES):
        y[c // 4] += res.results[c]["y"]
    y += b_out[None, None, :]
    return y


# revision 32
# speedup vs baseline: 1.0305x; 1.0305x over previous
"""Causal self-attention (B=2, T=4096, C=768, H=12, Dh=64) on 8 TRN2 NeuronCores.

Sharding: batch x head-groups. Core c handles batch b = c//4 and the 3 heads
hh = 3*(c%4) .. hh+2 of that batch (data parallel on B, tensor parallel on
heads for the qkv / out projections). Each core computes a partial output
y_c = attn_out(heads) @ W_out[head rows]; the host sums the 4 partials per
batch and adds b_out.

All matmul operands are bf16 (host pre-converts x / W_qkv / W_out); PSUM
accumulation stays fp32, y is written fp32. bf16 halves the input DMA and
runs matmuls at 1 cycle/row.

Device-side layout (per core, identical SPMD program):
  xt    [768, 4096]  x[b].T bf16 (C on partitions)
  wqkv  [768, 576]   bf16, columns permuted to [q0 q1 | k0 k1 | q2 k2 | v0 v1 v2]
  bqkv  [576]        f32, same permutation
  wout  [192, 768]   bf16, rows for this core's heads
  y     [4096, 768]  f32 partial output (no b_out)

Projection phase produces bf16:
  A  = [q0|q1]^T  [128, T]   (head0 on partitions 0-63, head1 on 64-127)
  B0 = [k0|0], B1 = [0|k1], D = [k2|0]   zero-padded k tiles [128, T]
  Cc = [q2|k2]^T  [128, T]
  v_st [128, 32, 3, 65]      v in [token, d] layout per 128-token block,
                             col 64 = 1.0 (softmax row-sums for free)

K=64 matmuls run at HALF rate on real TRN2 (measured via probes.py; the
CoreSim cost model misses this), so every s^T contraction is zero-padded to
128 partitions: lhsT = padded k tile (the zero half kills the other head's
rows in the packed q rhs), rhs = A or Cc full-height. Same trick pads the
out-projection's at2/wo2 pair.

Attention per (q-super of 1024, head): s^T tiles [128 k, 512 q] via
matmul(lhsT=k_pad, rhs=q_packed) into PSUM, exp'd into bf16 pt tiles either
on ACT (exact exp, scale=1/8, no max subtraction: logits ~N(0,1)) or on DVE
via int16 Schraudolph (bitcast bf16, ~3% err; never on qs=0 where diagonal
mass dominates). A static greedy balancer with hardware-calibrated per-op
costs splits the exp chunks + evacuations between ACT and DVE. The PE
stream is software-pipelined: av matmuls for round kb are emitted after the
s matmuls of round kb+lag so the in-order PE never stalls on an exp, and
the pipeline-drain bubbles at head boundaries are filled with the previous
super's out-projection tiles. Supers run in order 1,2,3,0 so the ACT-bound
qs=0 overlaps super 3's out-projection.

av^T accumulates matmul(lhsT=v_aug, rhs=p^T) over k-blocks into two
[65, 512] PSUM halves; the low half stops accumulating early (last_r0) and
is evacuated while the high half still runs, hiding the PSUM WAR on the
next head. Row 64 is the softmax denominator; normalize per 512-col half
with reciprocal_approx_fast + gpsimd partition-broadcast + DVE multiply
into bf16 attnT. Out-projection contracts attnT (2x K=128 matmuls) with
wout, evacuating per 512-col half to the y staging tile.
"""

import math

import numpy as np

import concourse.bass as bass
import concourse.tile as tile
from concourse import bacc, mybir
from concourse.bass_utils import run_bass_kernel_spmd

F32 = mybir.dt.float32
F32R = mybir.dt.float32r
BF16 = mybir.dt.bfloat16
I16 = mybir.dt.int16
I32 = mybir.dt.int32

# Schraudolph exp constants, int16 domain (bf16 = top 16 bits of f32):
# exp(s*SCALE) ~= bitcast_bf16(int16(s*A16 + B16))
LOG2E = 1.4426950408889634
SCH_A16 = 128.0 * LOG2E  # * SCALE applied at use site
SCH_B16 = 128.0 * (127.0 - 0.04367744890362246)

T = 4096
C = 768
H = 12
DH = 64
HPC = 3  # heads per core
NCORES = 8
SUP = 1024  # q-super width
NSUP = T // SUP
KB = 128  # k-block
NKB = T // KB
CH = 512  # st psum chunk / exp granularity
SCALE = 1.0 / math.sqrt(DH)

TRACE = False
LAST_RESULT = None
_PROG = None


DEFAULT_CFG = {
    "lag": 2,  # av pipeline depth in rounds behind s
    "st_bufs": 6,
    "av_bufs": 2,
    "pt_bufs": 4,
    "nrm_bufs": 2,
    "dve_exp": 1,  # allow DVE int16-Schraudolph for non-diagonal chunks
    "pool_tt": 0,  # run SBUF-only mask/norm multiplies on gpsimd (Pool)
    "repeat": 1,  # hardware For_i repetitions of the body (benchmarking)
}

# estimated per-op engine costs (ns) for the static ACT/DVE balancer,
# calibrated against hardware microbenchmarks (probes.py): 512-col exp on
# ACT = 638ns, int16 Schraudolph on DVE = 637ns, bf16 tt on DVE = 346ns
_ACT_COL = 0.833
_DVE_COL = 1.0417
_ACT_PSUM = 212.0
_DVE_PSUM = 104.0
_DVE_SBUF = 80.0


class _Balancer:
    def __init__(self):
        self.act = 0.0
        self.dve = 0.0

    def pick(self, act_cost, dve_cost, act_dve_extra=0.0):
        """Pick the engine that finishes this op sooner; update its load.
        act_dve_extra: DVE-side cost incurred when the ACT variant is chosen
        (e.g. the maskadd that precedes an ACT diagonal exp)."""
        if self.act + act_cost <= self.dve + dve_cost:
            self.act += act_cost
            self.dve += act_dve_extra
            return "act"
        self.dve += dve_cost
        return "dve"


def build_program(debug=False, cfg=None):
    cfg = {**DEFAULT_CFG, **(cfg or {})}
    nc = bacc.Bacc("TRN2", target_bir_lowering=False, debug=False)
    xt_d = nc.dram_tensor("xt", [C, T], BF16, kind="ExternalInput").ap()
    wqkv_d = nc.dram_tensor("wqkv", [C, 576], BF16, kind="ExternalInput").ap()
    bqkv_d = nc.dram_tensor("bqkv", [576], F32, kind="ExternalInput").ap()
    wout_d = nc.dram_tensor("wout", [192, C], BF16, kind="ExternalInput").ap()
    y_d = nc.dram_tensor("y", [T, C], F32, kind="ExternalOutput").ap()

    with tile.TileContext(nc) as tc:
        with tc.tile_pool(name="res", bufs=1) as res:
            # K=64 matmuls run at HALF rate on real TRN2 hardware, so every
            # s^T lhsT is zero-padded to a full 128-partition contraction:
            # the zero half kills the other head's rows in the packed rhs.
            A = res.tile([128, T], BF16, tag="A")  # [q0 | q1] packed
            B0 = res.tile([128, T], BF16, tag="B0")  # [k0 | 0]
            B1 = res.tile([128, T], BF16, tag="B1")  # [0 | k1]
            Cc = res.tile([128, T], BF16, tag="Cc")  # [q2 | k2] packed
            D = res.tile([128, T], BF16, tag="D")  # [k2 | 0]
            v_st = res.tile([128, NKB, HPC, DH + 1], BF16, tag="v_st")
            wo01 = res.tile([128, C], BF16, tag="wo01")
            wo2 = res.tile([128, C], BF16, tag="wo2")  # rows 64-127 zero
            at01 = res.tile([128, T], BF16, tag="at01")  # [h0 d | h1 d] x q
            at2 = res.tile([128, T], BF16, tag="at2")  # rows 64-127 zero
            nc.vector.memset(B0[64:128, :], 0.0)
            nc.vector.memset(B1[0:64, :], 0.0)
            nc.gpsimd.memset(D[64:128, :], 0.0)
            nc.gpsimd.memset(at2[64:128, :], 0.0)
            nc.vector.memset(wo2[64:128, :], 0.0)

            # multiplicative causal mask for the diagonal 128-block
            # (1.0 where q >= k else 0.0), bf16 for 2x DVE
            mask01 = res.tile([128, KB], BF16, tag="mask")
            nc.gpsimd.memset(mask01[:], 1.0)
            nc.gpsimd.affine_select(
                out=mask01[:],
                in_=mask01[:],
                compare_op=mybir.AluOpType.is_ge,
                fill=0.0,
                base=0,
                pattern=[[1, KB]],
                channel_multiplier=-1,
            )
            # additive variant (0 / -1e5) for diagonal tiles kept on ACT
            maskadd = res.tile([128, KB], F32, tag="maskadd")
            nc.gpsimd.memset(maskadd[:], 0.0)
            nc.gpsimd.affine_select(
                out=maskadd[:],
                in_=maskadd[:],
                compare_op=mybir.AluOpType.is_ge,
                fill=-1e5,
                base=0,
                pattern=[[1, KB]],
                channel_multiplier=-1,
            )
            nc.vector.memset(v_st[:, :, :, DH : DH + 1], 1.0)

            import contextlib

            rep_ctx = (
                tc.For_i(0, cfg["repeat"], 1)
                if cfg.get("repeat", 1) > 1
                else contextlib.nullcontext()
            )
            rep_ctx.__enter__()

            # ---------------- Phase 1: projections ----------------
            with (
                tc.tile_pool(name="p1", bufs=1) as p1,
                tc.tile_pool(name="xts", bufs=3) as xpool,
                tc.tile_pool(name="pps", bufs=2, space="PSUM") as pps,
                tc.tile_pool(name="vps", bufs=2, space="PSUM") as vps,
            ):
                wq_sb = p1.tile([128, 6, 576], BF16, tag="wq")
                bias_qk = p1.tile([128, 3], F32, tag="bqk")
                bias_v = p1.tile([128, 192], F32, tag="bv")
                bias_v_row = p1.tile([1, 192], F32, tag="bvr")

                def load_xts(ts):
                    xts = xpool.tile([128, 6, 512], BF16, tag="xts")
                    # two half-chunks: the ci=0..2 matmuls start after the
                    # first half lands
                    for half in range(2):
                        nc.sync.dma_start(
                            xts[:, half * 3 : (half + 1) * 3, :],
                            xt_d[
                                half * 384 : (half + 1) * 384,
                                ts * 512 : (ts + 1) * 512,
                            ].rearrange("(ci p) n -> p ci n", p=128),
                        )
                    return xts

                # startup order: weight chunk ci=0, then the first x chunk,
                # then the rest — the first matmul needs only ci=0 + x half 0
                nc.sync.dma_start(wq_sb[:, 0, :], wqkv_d[0:128, :])
                xts0 = load_xts(0)
                for ci in range(1, 6):
                    nc.sync.dma_start(
                        wq_sb[:, ci, :],
                        wqkv_d[ci * 128 : (ci + 1) * 128, :],
                    )
                nc.sync.dma_start(
                    bias_qk[:],
                    bqkv_d[0:384].rearrange("(m p) -> p m", p=128),
                )
                nc.sync.dma_start(
                    bias_v_row[0:1, :],
                    bqkv_d[384:576].rearrange("(b f) -> b f", b=1),
                )
                # wout is needed only in phase 2 — load it behind everything
                nc.sync.dma_start(wo01[:], wout_d[0:128, :])
                nc.sync.dma_start(wo2[0:64, :], wout_d[128:192, :])
                nc.gpsimd.partition_broadcast(bias_v[:], bias_v_row[0:1, :])

                for ts in range(T // 512):
                    xts = xts0 if ts == 0 else load_xts(ts)
                    col0 = ts * 512
                    # q/k rows (transposed layout): psum [qkv-rows, tokens]
                    for m in range(3):
                        psq = pps.tile([128, 512], F32, tag="psq")
                        for ci in range(6):
                            nc.tensor.matmul(
                                psq[:],
                                wq_sb[:, ci, m * 128 : (m + 1) * 128],
                                xts[:, ci, :],
                                start=(ci == 0),
                                stop=(ci == 5),
                            )
                        if m == 1:
                            # k0/k1 land in separate zero-padded tiles
                            nc.vector.tensor_scalar_add(
                                out=B0[0:64, col0 : col0 + 512],
                                in0=psq[0:64, :],
                                scalar1=bias_qk[0:64, m : m + 1],
                            )
                            nc.vector.tensor_scalar_add(
                                out=B1[64:128, col0 : col0 + 512],
                                in0=psq[64:128, :],
                                scalar1=bias_qk[64:128, m : m + 1],
                            )
                        else:
                            dest = A if m == 0 else Cc
                            nc.vector.tensor_scalar_add(
                                out=dest[:, col0 : col0 + 512],
                                in0=psq[:],
                                scalar1=bias_qk[:, m : m + 1],
                            )
                    # v in [token, d] layout: psum [tokens, 3*64]
                    for tb in range(4):
                        psv = vps.tile([128, 192], F32, tag="psv")
                        for ci in range(6):
                            nc.tensor.matmul(
                                psv[:],
                                xts[:, ci, tb * 128 : (tb + 1) * 128],
                                wq_sb[:, ci, 384:576],
                                start=(ci == 0),
                                stop=(ci == 5),
                            )
                        kb = ts * 4 + tb
                        nc.vector.tensor_tensor(
                            out=v_st[:, kb, :, 0:DH],
                            in0=psv[:].rearrange("p (h d) -> p h d", h=HPC),
                            in1=bias_v[:].rearrange("p (h d) -> p h d", h=HPC),
                            op=mybir.AluOpType.add,
                        )

            # k2 lives at partitions 64-127 of Cc; shift it down to D's rows
            # 0-63 (rows 64-127 of D stay zero) so h2's s matmul contracts a
            # full 128 partitions against the packed Cc rhs.
            nc.sync.dma_start(D[0:64, :], Cc[64:128, :])

            # ---------------- Phase 2: attention + out-projection ----------------
            bal = _Balancer()
            with (
                tc.tile_pool(name="stps", bufs=cfg["st_bufs"], space="PSUM") as stps,
                tc.tile_pool(name="avps", bufs=cfg["av_bufs"], space="PSUM") as avps,
                tc.tile_pool(name="ptp", bufs=cfg["pt_bufs"]) as ptp,
                tc.tile_pool(name="nrm", bufs=cfg["nrm_bufs"]) as nrm,
                tc.tile_pool(name="ysb", bufs=4) as ypool,
            ):

                def q_ap(h, kb):
                    # packed rhs: the other head's rows are killed by the
                    # zero half of the padded k lhsT
                    return Cc[:, :] if h == 2 else A[:, :]

                def k_ap(h, kb):
                    return (B0, B1, D)[h][:, :]

                def s_exp_round(qs, h, kb):
                    """s^T matmuls + exp into a bf16 pt tile for one
                    (q-super, head, k-block) round. Returns (pt, ext0)."""
                    q0 = qs * SUP
                    t = kb - qs * (SUP // KB)  # >= 0 on the diagonal
                    ext0 = max(t, 0) * KB
                    pt = ptp.tile([128, SUP], BF16, tag="pt")
                    for ch0 in range(0, SUP, CH):
                        ch1 = ch0 + CH
                        lo = max(ch0, ext0)
                        if lo >= ch1:
                            continue
                        n = ch1 - lo
                        st = stps.tile([128, CH], F32, tag="st")
                        nc.tensor.matmul(
                            st[:, lo - ch0 : CH],
                            k_ap(h, kb)[:, kb * KB : (kb + 1) * KB],
                            q_ap(h, kb)[:, q0 + lo : q0 + ch1],
                            start=True,
                            stop=True,
                        )
                        has_diag = t >= 0 and ch0 <= ext0 < ch1
                        if t >= 0 and qs == 0:
                            # qs=0: exact exp on ACT only (softmax there is
                            # mostly diagonal mass; Schraudolph's ~3% error
                            # would survive). Additive mask before exp.
                            eng = "act"
                        elif not cfg["dve_exp"]:
                            eng = "act"
                        else:
                            act_cost = n * _ACT_COL + _ACT_PSUM
                            dve_cost = n * _DVE_COL + _DVE_PSUM
                            mask_cost = 0.0
                            if has_diag:
                                # ACT variant needs a DVE maskadd; DVE variant
                                # a bf16 mask multiply
                                mask_cost = KB * _DVE_COL + _DVE_PSUM
                                dve_cost += KB * _DVE_COL * 0.5 + _DVE_SBUF
                            eng = bal.pick(act_cost, dve_cost, mask_cost)
                        if eng == "act":
                            if has_diag:
                                nc.vector.tensor_tensor(
                                    out=st[:, ext0 - ch0 : ext0 - ch0 + KB],
                                    in0=st[:, ext0 - ch0 : ext0 - ch0 + KB],
                                    in1=maskadd[:],
                                    op=mybir.AluOpType.add,
                                )
                            if t >= 0 and qs == 0:
                                bal.act += n * _ACT_COL + _ACT_PSUM
                                if has_diag:
                                    bal.dve += KB * _DVE_COL + _DVE_PSUM
                            nc.scalar.activation(
                                out=pt[:, lo:ch1],
                                in_=st[:, lo - ch0 : CH],
                                func=mybir.ActivationFunctionType.Exp,
                                bias=0.0,
                                scale=SCALE,
                            )
                        else:
                            nc.vector.tensor_scalar(
                                out=pt[:, lo:ch1].bitcast(I16),
                                in0=st[:, lo - ch0 : CH],
                                scalar1=float(SCH_A16 * SCALE),
                                scalar2=float(SCH_B16),
                                op0=mybir.AluOpType.mult,
                                op1=mybir.AluOpType.add,
                            )
                            if has_diag:
                                # SBUF-only multiply — gpsimd (Pool) keeps it
                                # off the exp engines when pool_tt is set
                                eng2 = nc.gpsimd if cfg["pool_tt"] else nc.vector
                                if not cfg["pool_tt"]:
                                    bal.dve += KB * _DVE_COL * 0.5 + _DVE_SBUF
                                eng2.tensor_tensor(
                                    out=pt[:, ext0 : ext0 + KB],
                                    in0=pt[:, ext0 : ext0 + KB],
                                    in1=mask01[:],
                                    op=mybir.AluOpType.mult,
                                )
                    return pt, ext0

                def av_round(qs, h, kb, pt, ext0, avA, avB, stg):
                    """Accumulate av^T += v_aug^T @ p^T for one round. av is
                    split into two [65,512] PSUM tiles; each half is
                    evacuated to stg as soon as its accumulation stops (the
                    low half stops early), hiding the WAR on the next head's
                    av tiles."""
                    nkb = (qs + 1) * (SUP // KB)
                    last_r0 = qs * 8 + 512 // KB - 1  # last kb touching [0,512)
                    c = ext0
                    while c < SUP:
                        ce = min((c // 512 + 1) * 512, SUP)
                        half = c // 512
                        av = avA if half == 0 else avB
                        stop_kb = last_r0 if ce <= 512 else nkb - 1
                        nc.tensor.matmul(
                            av[:, c - half * 512 : ce - half * 512],
                            v_st[:, kb, h, :],
                            pt[:, c:ce],
                            start=(kb == 0),
                            stop=(kb == stop_kb),
                        )
                        if kb == stop_kb:
                            n = 512
                            eng = bal.pick(
                                n * _ACT_COL + _ACT_PSUM, n * _DVE_COL + _DVE_PSUM
                            )
                            dst = stg[:, half * 512 : (half + 1) * 512]
                            if eng == "act":
                                nc.scalar.copy(dst, av[:])
                            else:
                                nc.vector.tensor_copy(dst, av[:])
                        c = ce

                def norm_head(qs, h, stg):
                    """rows 0-63 of stg divided by row 64, into attnT storage.
                    reciprocal_approx_fast misreads PSUM at partition offset
                    64, so this works on the SBUF stage; the denominator row
                    is DMA-shifted to partition 0 first."""
                    q0 = qs * SUP
                    l0 = nrm.tile([1, SUP], F32, tag="l0")
                    rec = nrm.tile([1, SUP], F32, tag="rec")
                    recb = nrm.tile([64, SUP], F32, tag="recb")
                    if h == 1:
                        # h1 rows belong at partitions 64-127 of at01; DVE
                        # can't shift partitions, so stage + DMA.
                        h1s = nrm.tile([64, SUP], BF16, tag="h1stage")
                    # per-512-half chain so the first at01 half (and with it
                    # the first out-projection tiles) unblocks early
                    for c0 in (0, SUP // 2):
                        c1 = c0 + SUP // 2
                        nc.sync.dma_start(l0[0:1, c0:c1], stg[64:65, c0:c1])
                        nc.vector.reciprocal_approx_fast(
                            out=rec[0:1, c0:c1], in_=l0[0:1, c0:c1]
                        )
                        nc.gpsimd.partition_broadcast(
                            recb[:, c0:c1], rec[0:1, c0:c1]
                        )
                        if h == 0:
                            dest = at01[0:64, q0 + c0 : q0 + c1]
                        elif h == 2:
                            dest = at2[0:64, q0 + c0 : q0 + c1]
                        else:
                            dest = h1s[:, c0:c1]
                        # SBUF-only multiply — gpsimd, keeping DVE free for exps
                        eng2 = nc.gpsimd if cfg["pool_tt"] else nc.vector
                        eng2.tensor_tensor(
                            out=dest,
                            in0=stg[0:64, c0:c1],
                            in1=recb[:, c0:c1],
                            op=mybir.AluOpType.mult,
                        )
                        if h == 1:
                            nc.sync.dma_start(
                                at01[64:128, q0 + c0 : q0 + c1], h1s[:, c0:c1]
                            )
                    bal.dve += SUP * _DVE_COL + 2 * _DVE_PSUM + 4 * _DVE_SBUF

                def oproj_tb(qs, tb):
                    """One 128-token tile of the out-projection for super qs.
                    Each 512-col half is evacuated and DMA'd independently so
                    the PSUM tile and y_sb recycle fast."""
                    tcol = qs * SUP + tb * 128
                    y_sb = ypool.tile([128, C], F32, tag="ysb")
                    for rs, re in ((0, 512), (512, C)):
                        yps = stps.tile([128, CH], F32, tag="st")
                        nc.tensor.matmul(
                            yps[:, 0 : re - rs],
                            at01[:, tcol : tcol + 128],
                            wo01[:, rs:re],
                            start=True,
                            stop=False,
                        )
                        nc.tensor.matmul(
                            yps[:, 0 : re - rs],
                            at2[:, tcol : tcol + 128],
                            wo2[:, rs:re],
                            start=False,
                            stop=True,
                        )  # at2 rows 64-127 are zero: full-K at 1c/row
                        n = re - rs
                        eng = bal.pick(
                            n * _ACT_COL + _ACT_PSUM, n * _DVE_COL + _DVE_PSUM
                        )
                        if eng == "act":
                            nc.scalar.copy(y_sb[:, rs:re], yps[:, 0 : re - rs])
                        else:
                            nc.vector.tensor_copy(y_sb[:, rs:re], yps[:, 0 : re - rs])
                        nc.sync.dma_start(
                            y_d[tcol : tcol + 128, rs:re], y_sb[:, rs:re]
                        )

                # Supers processed in order 1,2,3,0: qs=0 is PE-light but
                # ACT-bound (exact-exp only), so overlapping it with super
                # 3's out-projection keeps the PE fed and shrinks the tail.
                # Out-projection tiles of the previously processed super are
                # slotted after each head (fills the PE bubble while the norm
                # chain and the last exps drain).
                SUPER_ORDER = (1, 2, 3, 0)
                OPROJ_SLOTS = {0: (0, 1, 2), 1: (3, 4, 5), 2: (6, 7)}
                LAG = cfg["lag"]

                for i, qs in enumerate(SUPER_ORDER):
                    prev = SUPER_ORDER[i - 1] if i >= 1 else None
                    nkb = (qs + 1) * (SUP // KB)
                    for h in range(HPC):
                        avA = avps.tile([65, SUP // 2], F32, tag="av")
                        avB = avps.tile([65, SUP // 2], F32, tag="av")
                        stg = nrm.tile([65, SUP], F32, tag="avstage")
                        slots = list(OPROJ_SLOTS[h]) if prev is not None else []
                        pend = []
                        for kb in range(nkb):
                            pend.append((kb, *s_exp_round(qs, h, kb)))
                            if len(pend) > LAG:
                                kb0, pt0, e0 = pend.pop(0)
                                av_round(qs, h, kb0, pt0, e0, avA, avB, stg)
                        while pend:
                            kb0, pt0, e0 = pend.pop(0)
                            av_round(qs, h, kb0, pt0, e0, avA, avB, stg)
                            # fill the pipeline-drain bubble (the last avs
                            # wait on exps) with out-projection PE work
                            if slots:
                                oproj_tb(prev, slots.pop(0))
                        norm_head(qs, h, stg)
                        for tb in slots:
                            oproj_tb(prev, tb)
                for tb in range(SUP // 128):
                    oproj_tb(SUPER_ORDER[-1], tb)
            rep_ctx.__exit__(None, None, None)

    nc.compile()
    return nc


def shard_inputs(x, W_qkv, b_qkv, W_out, b_out):
    """Build the per-core input maps (host-side sharding, bf16 weights/x)."""
    import ml_dtypes

    bf16 = ml_dtypes.bfloat16
    x = np.asarray(x, dtype=np.float32)
    W_qkv = np.asarray(W_qkv, dtype=np.float32)
    b_qkv = np.asarray(b_qkv, dtype=np.float32)
    W_out = np.asarray(W_out, dtype=np.float32)
    in_maps = []
    for c in range(NCORES):
        b = c // 4
        hh = (c % 4) * HPC
        h0, h1, h2 = hh, hh + 1, hh + 2

        def qcols(h):
            return list(range(h * DH, (h + 1) * DH))

        def kcols(h):
            return list(range(C + h * DH, C + (h + 1) * DH))

        def vcols(h):
            return list(range(2 * C + h * DH, 2 * C + (h + 1) * DH))

        perm = (
            qcols(h0) + qcols(h1) + kcols(h0) + kcols(h1) + qcols(h2) + kcols(h2)
            + vcols(h0) + vcols(h1) + vcols(h2)
        )
        in_maps.append(
            {
                "xt": np.ascontiguousarray(x[b].T).astype(bf16),
                "wqkv": np.ascontiguousarray(W_qkv[:, perm]).astype(bf16),
                "bqkv": np.ascontiguousarray(b_qkv[perm]),
                "wout": np.ascontiguousarray(
                    W_out[hh * DH : (hh + HPC) * DH, :]
                ).astype(bf16),
            }
        )
    return in_maps


def kernel(x, W_qkv, b_qkv, W_out, b_out):
    global _PROG, LAST_RESULT
    if _PROG is None:
        _PROG = build_program()
    nc = _PROG
    in_maps = shard_inputs(x, W_qkv, b_qkv, W_out, b_out)
    res = run_bass_kernel_spmd(nc, in_maps, list(range(NCORES)), trace=TRACE)
    LAST_RESULT = res
    b_out = np.asarray(b_out, dtype=np.float32)
    y = np.zeros((2, T, C), dtype=np.float32)
    for c in range(NCORES):
        y[c // 4] += res.results[c]["y"]
    y += b_out[None, None, :]
    return y
